# revision 1
# baseline (speedup 1.0000x reference)
"""Trainium2 Bass kernel for 3D windowed (3x3x3) per-channel softmax attention.

Problem (hardcoded): x (1,32,24,48,64) f32; Wq/Wk/Wv (48,32); rel_* (16,...,3).
  q = Wq@x ; kf/vf = Wk/Wv @ pad(x) ; per (c,voxel): softmax over the 27
  window taps of q*(k_win+rel), then weighted sum of v_win.

Strategy:
  - Shard D=24 across 8 cores (3 output slices each + 1-voxel halo, zero-padded
    on host). Each core is fully independent (SPMD, no collectives).
  - Host pre-tiles each core's padded slab into 8 overlapping H-blocks
    (block-major layout) so every bulk DMA on chip is contiguous.
  - On-chip rows r = (s, c): s in [0,8) H-blocks x c in [0,48) channels:
    384 rows = 3 passes x 128 partitions (100% partition utilization).
  - Per pass: kf/vf tiles [128, 5, 8, 66] bf16, q [128, 3, 6, 64] bf16.
    For each of 27 window offsets j: one scalar_tensor_tensor computes
    (kf_shifted + rel[c,j]) * q, ACT computes exp, one tensor_tensor computes
    e * vf_shifted; pairwise trees reduce the 27 e/wv planes; approx
    reciprocal + mul finishes softmax. No max-subtraction (|logits| <~ 10,
    exp is safe in f32).
"""

import sys

sys.path.insert(0, "/opt/trn_rl_repo")

import numpy as np

import concourse.bass as bass
import concourse.bacc as bacc
import concourse.mybir as mybir
import concourse.tile as tile
from concourse.bass_utils import run_bass_kernel_spmd

# ---- problem constants (hardcoded per contract) ----
B, CIN, D, H, W = 1, 32, 24, 48, 64
COUT, K, C3 = 48, 3, 16
NCORES = 8
DLOC = D // NCORES            # 3 output d-slices per core
DP = DLOC + 2                 # 5 padded d-planes per core
NS = 8                        # H-blocks per core
HB = H // NS                  # 6 output rows per block
HBP = HB + 2                  # 8 padded rows per block
WP = W + 2                    # 66
NROW = NS * COUT              # 384 rows = (s, c)
NPASS = NROW // 128           # 3 passes of 128 partitions
BLK = DP * HBP * WP           # 2640 padded voxels per block
VOXP = DLOC * HB * W          # 1152 output voxels per row-block
NJ = K * K * K                # 27

F32 = mybir.dt.float32
BF16 = mybir.dt.bfloat16

_CACHE = {}


def _pass_spans():
    """Per pass: (row0, row1, s, c0, c1) spans with constant s."""
    spans = []
    for p in range(NPASS):
        lo, hi = 128 * p, 128 * (p + 1)
        out, r = [], lo
        while r < hi:
            s, c0 = divmod(r, COUT)
            c1 = min(COUT, c0 + (hi - r))
            out.append((r - lo, r - lo + (c1 - c0), s, c0, c1))
            r += c1 - c0
        spans.append(out)
    return spans


def build_program(chunk_h=3):
    nc = bacc.Bacc("TRN2", target_bir_lowering=False, debug=False, num_devices=NCORES)

    xs = nc.declare_dram_parameter("xs", [CIN, NS, DP, HBP, WP], BF16, isOutput=False)
    wq = nc.declare_dram_parameter("wq", [CIN, COUT], BF16, isOutput=False)
    wkv = nc.declare_dram_parameter("wkv", [CIN, 112], BF16, isOutput=False)
    relt = nc.declare_dram_parameter("relt", [NPASS, 128, NJ], F32, isOutput=False)
    y = nc.declare_dram_parameter("y", [NPASS, 128, DLOC, HB, W], F32, isOutput=True)
    kvdr = nc.dram_tensor("kvdr", [112, NS, DP, HBP, WP], BF16)
    qdr = nc.dram_tensor("qdr", [COUT, NS, DLOC, HB, WP], BF16)

    spans = _pass_spans()
    n_hc = HB // chunk_h
    VCH = DLOC * chunk_h * W

    with tile.TileContext(nc) as tc:
        with (
            tc.tile_pool(name="consts", bufs=1) as consts,
            tc.tile_pool(name="psum", bufs=8, space="PSUM") as psum_pool,
            tc.tile_pool(name="rows", bufs=1) as rows_pool,
        ):
            # ---- tiny constants ----
            wq_sb = consts.tile([CIN, COUT], BF16, name="wq_sb")
            nc.sync.dma_start(out=wq_sb, in_=wq[:])
            wkv_sb = consts.tile([CIN, 112], BF16, name="wkv_sb")
            nc.sync.dma_start(out=wkv_sb, in_=wkv[:])
            rel_sb = consts.tile([128, NPASS, NJ], F32)
            for p in range(NPASS):
                nc.sync.dma_start(out=rel_sb[:, p], in_=relt[p])

            # ---- per-pass row tiles (live through the whole kernel) ----
            kf = [rows_pool.tile([128, DP, HBP, WP], BF16, tag=f"kf{p}", name=f"kf{p}")
                  for p in range(NPASS)]
            vf = [rows_pool.tile([128, DP, HBP, WP], BF16, tag=f"vf{p}", name=f"vf{p}")
                  for p in range(NPASS)]
            qt = [rows_pool.tile([128, DLOC, HB, WP], BF16, tag=f"qt{p}", name=f"qt{p}")
                  for p in range(NPASS)]

            # ---- load x (block-major), project on PE, stage, scatter ----
            with tc.tile_pool(name="xstage", bufs=1) as stage_pool:
                x_sb = stage_pool.tile([CIN, NS, DP, HBP, WP], BF16, name="x_sb")
                nc.sync.dma_start(out=x_sb, in_=xs[:])
                kvst = stage_pool.tile([112, NS, DP, HBP, WP], BF16, tag="kvst", name="kvst")
                qst = stage_pool.tile([COUT, NS, DLOC, HB, WP], BF16, tag="qst", name="qst")
                nc.vector.memset(qst, 0.0)

                for s in range(NS):
                    xb = x_sb[:, s].rearrange("c d h w -> c (d h w)")
                    # k+v fused: one [32,112] matmul per chunk (v at rows 64+)
                    stb = kvst[:, s].rearrange("c d h w -> c (d h w)")
                    for i in range(BLK // 440):
                        ps = psum_pool.tile([112, 440], F32, tag="ps", name="ps")
                        nc.tensor.matmul(
                            ps, wkv_sb, xb[:, i * 440:(i + 1) * 440],
                            start=True, stop=True)
                        nc.scalar.copy(stb[:, i * 440:(i + 1) * 440], ps)
                    # q over the block interior: 3 chunks of 384 (2h-rows each)
                    for i in range(3):
                        ps = psum_pool.tile([COUT, 384], F32, tag="ps", name="ps2")
                        rhs = x_sb[:, s, 1:1 + DLOC, 1 + 2 * i:1 + 2 * i + 2, 1:1 + W]
                        nc.tensor.matmul(ps, wq_sb, rhs, start=True, stop=True)
                        nc.scalar.copy(
                            qst[:, s, :, 2 * i:2 * i + 2, 0:W], ps)

                # bounce stages through DRAM, then gather per-pass rows
                nc.sync.dma_start(out=kvdr[:], in_=kvst)
                nc.sync.dma_start(out=qdr[:], in_=qst)
                for p in range(NPASS):
                    for (r0, r1, s, c0, c1) in spans[p]:
                        nc.sync.dma_start(out=kf[p][r0:r1], in_=kvdr[c0:c1, s])
                        nc.sync.dma_start(out=vf[p][r0:r1], in_=kvdr[64 + c0:64 + c1, s])
                        nc.sync.dma_start(out=qt[p][r0:r1], in_=qdr[c0:c1, s])

            # ---- attention ----
            # Flat (h,w) window reads: per j a single scalar_tensor_tensor
            # covers all 3 d-planes with a [d, 396]-span AP (the 2 pad
            # columns per row are computed as garbage and dropped at the
            # output). bf16 scratches; f32 only for final sums/reciprocal.
            with tc.tile_pool(name="attn", bufs=1) as attn, \
                 tc.tile_pool(name="outs", bufs=1) as outs:
                FL = (HB - 1) * WP + W       # 394-elem flat span per d-plane
                NV = DLOC * FL               # 1188 per scratch plane
                for p in range(NPASS):
                    out_t = outs.tile([128, DLOC, HB, W], F32, tag="out", name="out_t")
                    scr = attn.tile([128, 2, NJ, NV], BF16, tag="scr", name="scr")
                    fin = attn.tile([128, 3, NV], F32, tag="fin", name="fin")
                    kff = kf[p].rearrange("r d h w -> r d (h w)")
                    vff = vf[p].rearrange("r d h w -> r d (h w)")
                    qtf = qt[p].rearrange("r d h w -> r d (h w)")
                    for j in range(NJ):
                        dj, hj, wj = j // 9, (j // 3) % 3, j % 3
                        nc.vector.scalar_tensor_tensor(
                            out=scr[:, 0, j],
                            in0=kff[:, dj:dj + DLOC, hj * WP + wj:hj * WP + wj + FL],
                            scalar=rel_sb[:, p, j:j + 1], in1=qtf[:, :, 0:FL],
                            op0=mybir.AluOpType.add, op1=mybir.AluOpType.mult)
                        if j % 9 == 8:
                            g = j // 9  # = dj of this group
                            nc.scalar.activation(
                                out=scr[:, 0, 9 * g:9 * g + 9],
                                in_=scr[:, 0, 9 * g:9 * g + 9],
                                func=mybir.ActivationFunctionType.Exp)
                            for hj2 in range(K):
                                j0 = 9 * g + 3 * hj2
                                base = vff[:, g, hj2 * WP:hj2 * WP + 1]
                                vwin = bass.AP(
                                    tensor=base.tensor,
                                    offset=base.offset,
                                    ap=[base.ap[0], [1, K], [HBP * WP, DLOC],
                                        [1, FL]])
                                nc.vector.tensor_tensor(
                                    out=scr[:, 1, j0:j0 + K],
                                    in0=scr[:, 0, j0:j0 + K],
                                    in1=vwin, op=mybir.AluOpType.mult)
                    # pairwise tree over 27 planes for both halves at once
                    nc.vector.tensor_tensor(
                        out=scr[:, :, 0:13], in0=scr[:, :, 0:26:2],
                        in1=scr[:, :, 1:27:2], op=mybir.AluOpType.add)
                    nc.vector.tensor_tensor(
                        out=scr[:, :, 0:6], in0=scr[:, :, 0:12:2],
                        in1=scr[:, :, 1:13:2], op=mybir.AluOpType.add)
                    nc.vector.tensor_tensor(
                        out=scr[:, :, 0:3], in0=scr[:, :, 0:6:2],
                        in1=scr[:, :, 1:7:2], op=mybir.AluOpType.add)
                    nc.vector.tensor_tensor(
                        out=scr[:, :, 0:1], in0=scr[:, :, 0:1],
                        in1=scr[:, :, 1:2], op=mybir.AluOpType.add)
                    nc.vector.tensor_tensor(
                        out=scr[:, :, 1:2], in0=scr[:, :, 2:3],
                        in1=scr[:, :, 12:13], op=mybir.AluOpType.add)
                    nc.vector.tensor_tensor(
                        out=scr[:, :, 0:1], in0=scr[:, :, 0:1],
                        in1=scr[:, :, 1:2], op=mybir.AluOpType.add)
                    # final level in f32: fin[0]=sum(e), fin[1]=sum(e*v)
                    nc.vector.tensor_tensor(
                        out=fin[:, 0:2], in0=scr[:, :, 0],
                        in1=scr[:, :, 26], op=mybir.AluOpType.add)
                    nc.vector.reciprocal_approx_accurate(
                        out=fin[:, 2], in_=fin[:, 0],
                        scratch=scr[:, 0, 0:2].rearrange("r a b -> r (a b)").bitcast(F32))
                    def fin_dhw(a):
                        b = fin[:, a, 0:1]
                        return bass.AP(tensor=b.tensor, offset=b.offset,
                                       ap=[b.ap[0], [FL, DLOC], [WP, HB], [1, W]])
                    nc.vector.tensor_tensor(
                        out=out_t, in0=fin_dhw(1), in1=fin_dhw(2),
                        op=mybir.AluOpType.mult)
                    nc.gpsimd.dma_start(out=y[p], in_=out_t)
    nc.compile()
    return nc


def _host_prep(x, Wq, Wk, Wv, rel_h, rel_w, rel_d):
    x = np.asarray(x, np.float32).reshape(CIN, D, H, W)
    xp = np.pad(x, ((0, 0), (1, 1), (1, 1), (1, 1)))  # (32, 26, 50, 66)
    wqT = np.ascontiguousarray(np.asarray(Wq, np.float32).T)
    wkvT = np.zeros((CIN, 112), np.float32)
    wkvT[:, 0:48] = np.asarray(Wk, np.float32).T
    wkvT[:, 64:112] = np.asarray(Wv, np.float32).T
    bf = np.dtype("bfloat16") if hasattr(np, "bfloat16") else None

    import ml_dtypes
    tobf = lambda a: a.astype(ml_dtypes.bfloat16)

    # rel table: relfull[c, j], j = dj*9 + hj*3 + wj
    rel_d2 = np.asarray(rel_d, np.float32).reshape(C3, K)  # varies over wj
    rel_h2 = np.asarray(rel_h, np.float32).reshape(C3, K)  # varies over dj
    rel_w2 = np.asarray(rel_w, np.float32).reshape(C3, K)  # varies over hj
    relfull = np.zeros((COUT, NJ), np.float32)
    for j in range(NJ):
        dj, hj, wj = j // 9, (j // 3) % 3, j % 3
        relfull[0:16, j] = rel_d2[:, wj]
        relfull[16:32, j] = rel_h2[:, dj]
        relfull[32:48, j] = rel_w2[:, hj]
    rows = np.arange(NROW) % COUT
    relt = np.ascontiguousarray(relfull[rows].reshape(NPASS, 128, NJ))

    in_maps = []
    for i in range(NCORES):
        slab = xp[:, 3 * i:3 * i + DP]  # (32, 5, 50, 66)
        # block-major: (CIN, NS, DP, HBP, WP)
        xb = np.empty((CIN, NS, DP, HBP, WP), np.float32)
        for s in range(NS):
            xb[:, s] = slab[:, :, HB * s:HB * s + HBP, :]
        in_maps.append({
            "xs": tobf(xb), "wq": tobf(wqT), "wkv": tobf(wkvT),
            "relt": relt,
        })
    return in_maps


def kernel(x, Wq, Wk, Wv, rel_h, rel_w, rel_d, trace=False):
    in_maps = _host_prep(x, Wq, Wk, Wv, rel_h, rel_w, rel_d)
    if "nc" not in _CACHE:
        _CACHE["nc"] = build_program()
    res = run_bass_kernel_spmd(
        _CACHE["nc"], in_maps, core_ids=list(range(NCORES)), trace=trace)
    # y per core: (NPASS, 128, DLOC, HB, W); row r=(s,c) -> out[c, :, 6s:6s+6, :]
    out = np.zeros((COUT, D, H, W), np.float32)
    rows = np.arange(NROW)
    ss, cs = rows // COUT, rows % COUT
    for i in range(NCORES):
        yv = np.asarray(res.results[i]["y"]).reshape(NROW, DLOC, HB, W)
        for s in range(NS):
            sel = yv[ss == s]  # (48, DLOC, HB, W) ordered by c
            out[:, 3 * i:3 * i + DLOC, HB * s:HB * s + HB, :] = sel
    if trace:
        _CACHE["last"] = res
    return out.reshape(1, COUT, D, H, W)



# revision 8
# speedup vs baseline: 1.5579x; 1.5579x over previous
"""Trainium2 Bass kernel for 3D windowed (3x3x3) per-channel softmax attention.

Problem (hardcoded): x (1,32,24,48,64) f32; Wq/Wk/Wv (48,32); rel_* (16,...,3).
  q = Wq@x ; kf/vf = Wk/Wv @ pad(x) ; per (c,voxel): softmax over the 27
  window taps of q*(k_win+rel), then weighted sum of v_win.

Strategy (v2):
  - Shard D=24 across 8 cores (3 output d-slices each + 1-voxel halo,
    zero-padded on host). SPMD, no collectives.
  - Rows r = (s, c_sub): 8 H-blocks x 16 channels = 128 partitions/pass,
    3 passes = one channel GROUP of 16 per pass. Within a pass every
    channel shares the same rel axis (ch 0-15: rel_d varies over wj,
    16-31: rel_h over dj, 32-47: rel_w over hj), so
      exp(q*(k+rel_j)) = exp(q*k_j) * F_a,   F_a = exp(q*rel_a)
    factors with only 3 F planes per pass.
  - qk logits via plain tensor_tensor (2x DVE mode; the baseline's
    scalar_tensor_tensor runs at 1x), batched 3 taps per op via window APs.
  - exp on ACT (in-place over the logit planes).
  - Per-axis sums S_a = sum_{j in a} e_j and T_a = sum_{j in a} e_j*v_j
    either on PE (identity-matmul PSUM accumulation, Pool evicts) or on
    DVE (pairwise trees) - configurable via RED_DEN/RED_NUM.
  - den = sum_a F_a*S_a, num = sum_a F_a*T_a, out = num * recip(den).
  - No DRAM bounce: projections go PSUM -> SBUF stage -> SBUF-SBUF DMA
    gather into per-pass row tiles.
"""

import sys

sys.path.insert(0, "/opt/trn_rl_repo")

import numpy as np

import concourse.bass as bass
import concourse.bacc as bacc
import concourse.mybir as mybir
import concourse.tile as tile
from concourse.bass_utils import run_bass_kernel_spmd

# ---- problem constants (hardcoded per contract) ----
B, CIN, D, H, W = 1, 32, 24, 48, 64
COUT, K, C3 = 48, 3, 16
NCORES = 8
DLOC = D // NCORES            # 3 output d-slices per core
DP = DLOC + 2                 # 5 padded d-planes per core
NS = 8                        # H-blocks per core
HB = H // NS                  # 6 output rows per block
HBP = HB + 2                  # 8 padded rows per block
WP = W + 2                    # 66
BLK = DP * HBP * WP           # 2640 padded voxels per block
SL = HBP * WP                 # 528: one padded d-plane
FL = (HB - 1) * WP + W        # 394-elem flat (h,w) span per d-plane
NV = DLOC * FL                # 1182 per scratch plane
NPASS = 3                     # one channel group per pass
CG = 16                       # channels per group
NJ = 27
NOUT = DLOC * HB * W          # 1152 true output voxels per row

# reduction engines: "pe" (identity matmul accumulate) or "dve" (pair tree)
RED_DEN = "pe"
RED_NUM = "pe"

F32 = mybir.dt.float32
BF16 = mybir.dt.bfloat16

_CACHE = {}


def _tap_geometry(g):
    """Per pass g: list over groups a of list of 3 qk/ev ops.

    Each op is (slot_base, koff, bstride) covering slots
    [slot_base, slot_base+3) with window offsets koff + i*bstride.
    Slot order per pass puts the rel axis outermost (slot = a*9 + o2*3 + i).
    """
    ops = []
    for a in range(3):
        row = []
        for o2 in range(3):
            if g == 0:    # a=wj, o2=dj, batch=hj
                dj, hj, wj, bs = o2, 0, a, WP
            elif g == 1:  # a=dj, o2=hj, batch=wj
                dj, hj, wj, bs = a, o2, 0, 1
            else:         # a=hj, o2=dj, batch=wj
                dj, hj, wj, bs = o2, a, 0, 1
            row.append((a * 9 + o2 * 3, dj * SL + hj * WP + wj, bs))
        ops.append(row)
    return ops


def _win_ap(flat, off, bstride):
    """[128, 3(batch), 3(d), 394] window view into a [128, 2640] tile."""
    base = flat[:, off:off + 1]
    return bass.AP(tensor=base.tensor, offset=base.offset,
                   ap=[base.ap[0], [bstride, 3], [SL, DLOC], [1, FL]])


def _q_ap(qt, rep):
    """[128, rep(broadcast), 3(d), 394] from a [128, NV] q tile."""
    base = qt[:, 0:1]
    return bass.AP(tensor=base.tensor, offset=base.offset,
                   ap=[base.ap[0], [0, rep], [FL, DLOC], [1, FL]])


def _scr_ap(scr, s0, n):
    """[128, n(slots), 3(d), 394] view of scratch slots [s0, s0+n)."""
    base = scr[:, s0, 0:1]
    return bass.AP(tensor=base.tensor, offset=base.offset,
                   ap=[base.ap[0], [NV, n], [FL, DLOC], [1, FL]])


def _dhw_ap(t, col0=0):
    """[128, 3(d), 6(h), 64(w)] true-voxel view of a [128, NV] plane tile."""
    base = t[:, col0:col0 + 1]
    return bass.AP(tensor=base.tensor, offset=base.offset,
                   ap=[base.ap[0], [FL, DLOC], [WP, HB], [1, W]])


def build_program():
    nc = bacc.Bacc("TRN2", target_bir_lowering=False, debug=False,
                   num_devices=NCORES)

    xs = nc.declare_dram_parameter("xs", [CIN, NS, DP, HBP, WP], BF16,
                                   isOutput=False)
    wq = nc.declare_dram_parameter("wq", [CIN, COUT], BF16, isOutput=False)
    wkv = nc.declare_dram_parameter("wkv", [CIN, 96], BF16, isOutput=False)
    relt = nc.declare_dram_parameter("relt", [NPASS, 128, 3], F32,
                                     isOutput=False)
    eye = nc.declare_dram_parameter("eye", [128, 128], BF16, isOutput=False)
    y = nc.declare_dram_parameter("y", [NPASS, 128, NOUT], BF16, isOutput=True)

    # psum column chunks for the PE reductions
    CH = [(0, 394), (394, 394), (788, 394)]

    with tile.TileContext(nc) as tc:
        with (
            tc.tile_pool(name="consts", bufs=1) as consts,
            tc.tile_pool(name="rows", bufs=1) as rows_pool,
            tc.tile_pool(name="attn", bufs=1) as attn,
        ):
            # ---- constants ----
            wq_sb = consts.tile([CIN, COUT], BF16, name="wq_sb")
            nc.sync.dma_start(out=wq_sb, in_=wq[:])
            wkv_sb = consts.tile([CIN, 96], BF16, name="wkv_sb")
            nc.sync.dma_start(out=wkv_sb, in_=wkv[:])
            rel_sb = consts.tile([128, NPASS, 3], F32, name="rel_sb")
            nc.sync.dma_start(out=rel_sb, in_=relt[:].rearrange("p r a -> r p a"))
            eye_sb = consts.tile([128, 128], BF16, name="eye_sb")
            nc.sync.dma_start(out=eye_sb, in_=eye[:])

            # ---- per-pass row tiles (all 3 passes resident) ----
            kf = [rows_pool.tile([128, BLK], BF16, name=f"kf{g}")
                  for g in range(NPASS)]
            vf = [rows_pool.tile([128, BLK], BF16, name=f"vf{g}")
                  for g in range(NPASS)]
            qt = [rows_pool.tile([128, NV], BF16, name=f"qt{g}")
                  for g in range(NPASS)]

            # ---- projection prologue, per s-block ----
            with (
                tc.tile_pool(name="psum", bufs=2, space="PSUM") as psum_pool,
                tc.tile_pool(name="stage", bufs=2) as stage,
            ):
              for s in range(NS):
                xst = stage.tile([CIN, BLK], BF16, tag="xst", name=f"xst{s}")
                nc.sync.dma_start(out=xst, in_=xs[:, s].rearrange(
                    "c d h w -> c (d h w)"))
                kvst = stage.tile([96, BLK], BF16, tag="kvst", name=f"kvst{s}")
                qst = stage.tile([COUT, DLOC, FL], BF16, tag="qst",
                                 name=f"qst{s}")
                for i in range(BLK // 440):
                    ps = psum_pool.tile([96, 440], F32, tag="pskv", name="pskv")
                    nc.tensor.matmul(ps, wkv_sb, xst[:, i * 440:(i + 1) * 440],
                                     start=True, stop=True)
                    nc.scalar.copy(kvst[:, i * 440:(i + 1) * 440], ps)
                # q on the block interior; one matmul per output d-plane
                for d in range(DLOC):
                    psq = psum_pool.tile([COUT, HB * W], F32, tag="psq",
                                         name="psq")
                    rb = xst[:, (d + 1) * SL + WP + 1:(d + 1) * SL + WP + 2]
                    rhs = bass.AP(tensor=rb.tensor, offset=rb.offset,
                                  ap=[rb.ap[0], [WP, HB], [1, W]])
                    nc.tensor.matmul(psq, wq_sb, rhs, start=True, stop=True)
                    qb = qst[:, d, 0:1]
                    qout = bass.AP(tensor=qb.tensor, offset=qb.offset,
                                   ap=[qb.ap[0], [WP, HB], [1, W]])
                    nc.scalar.copy(qout, psq)
                # scatter stage rows into per-pass row tiles (SBUF->SBUF DMA)
                r0 = s * CG
                for g in range(NPASS):
                    nc.sync.dma_start(out=kf[g][r0:r0 + CG],
                                      in_=kvst[g * CG:(g + 1) * CG])
                    nc.sync.dma_start(out=vf[g][r0:r0 + CG],
                                      in_=kvst[48 + g * CG:48 + (g + 1) * CG])
                    nc.sync.dma_start(
                        out=qt[g][r0:r0 + CG],
                        in_=qst[g * CG:(g + 1) * CG].rearrange(
                            "c d f -> c (d f)"))

            # ---- attention passes ----
            rpsum_ctx = tc.tile_pool(name="rpsum", bufs=2, space="PSUM")
            rpsum_pool = rpsum_ctx.__enter__()
            # scratch: per-group slot tiles so cross-pass deps stay fine-grained
            scrg = [attn.tile([128, 9, NV], BF16, name=f"scrg{a}")
                    for a in range(3)]
            scrF = attn.tile([128, 3, NV], BF16, name="scrF")
            tmp4 = attn.tile([128, 4, NV], BF16, name="tmp4")
            S_t = attn.tile([128, 3, NV], BF16, name="S_t")
            T_t = attn.tile([128, 3, NV], BF16, name="T_t")
            fs_t = attn.tile([128, 3, NV], BF16, name="fs_t")
            ft_t = attn.tile([128, 3, NV], BF16, name="ft_t")
            dsum = attn.tile([128, NV], BF16, name="dsum")
            nsum = attn.tile([128, NV], BF16, name="nsum")
            den32 = attn.tile([128, NV], F32, name="den32")
            rcp32 = attn.tile([128, NV], F32, name="rcp32")

            def pe_reduce(src_tile, dst, a):
                """dst[:, a] = sum of src_tile's 9 planes (identity matmul)."""
                for (c0, cw) in CH:
                    psr = rpsum_pool.tile([128, cw], F32, tag=f"psr{c0}",
                                          name="psr")
                    for j in range(9):
                        nc.tensor.matmul(
                            psr, eye_sb, src_tile[:, j, c0:c0 + cw],
                            start=(j == 0), stop=(j == 8))
                    nc.scalar.copy(dst[:, a, c0:c0 + cw], psr)

            def dve_reduce(src_tile, dst, a):
                """dst[:, a] = sum of src_tile's 9 planes (pair tree)."""
                nc.vector.tensor_tensor(
                    out=tmp4, in0=src_tile[:, 0:8:2], in1=src_tile[:, 1:8:2],
                    op=mybir.AluOpType.add)
                nc.vector.tensor_tensor(
                    out=tmp4[:, 0:2], in0=tmp4[:, 0:2], in1=tmp4[:, 2:4],
                    op=mybir.AluOpType.add)
                nc.vector.tensor_tensor(
                    out=tmp4[:, 0], in0=tmp4[:, 0], in1=tmp4[:, 1],
                    op=mybir.AluOpType.add)
                nc.vector.tensor_tensor(
                    out=dst[:, a], in0=tmp4[:, 0], in1=src_tile[:, 8],
                    op=mybir.AluOpType.add)

            for g in range(NPASS):
                geo = _tap_geometry(g)
                kff, vff, qtg = kf[g], vf[g], qt[g]
                outt = attn.tile([128, NOUT], BF16, tag="outt", name="outt")

                # logits: 3 groups x 3 TT ops, then F pre-mult planes
                for a in range(3):
                    for (sb, koff, bs) in geo[a]:
                        nc.vector.tensor_tensor(
                            out=_scr_ap(scrg[a], sb - a * 9, 3),
                            in0=_q_ap(qtg, 3),
                            in1=_win_ap(kff, koff, bs),
                            op=mybir.AluOpType.mult)
                    nc.vector.tensor_scalar(
                        out=scrF[:, a], in0=qtg,
                        scalar1=rel_sb[:, g, a:a + 1], scalar2=None,
                        op0=mybir.AluOpType.mult)
                # exp (in place)
                for a in range(3):
                    nc.scalar.activation(
                        out=scrg[a].rearrange("r j v -> r (j v)"),
                        in_=scrg[a].rearrange("r j v -> r (j v)"),
                        func=mybir.ActivationFunctionType.Exp)
                nc.scalar.activation(
                    out=scrF.rearrange("r j v -> r (j v)"),
                    in_=scrF.rearrange("r j v -> r (j v)"),
                    func=mybir.ActivationFunctionType.Exp)
                # denominator per-axis sums
                for a in range(3):
                    if RED_DEN == "pe":
                        pe_reduce(scrg[a], S_t, a)
                    else:
                        dve_reduce(scrg[a], S_t, a)
                # e <- e * v_win (in place), then numerator per-axis sums
                for a in range(3):
                    for (sb, koff, bs) in geo[a]:
                        ap = _scr_ap(scrg[a], sb - a * 9, 3)
                        nc.vector.tensor_tensor(
                            out=ap, in0=ap, in1=_win_ap(vff, koff, bs),
                            op=mybir.AluOpType.mult)
                    if RED_NUM == "pe":
                        pe_reduce(scrg[a], T_t, a)
                    else:
                        dve_reduce(scrg[a], T_t, a)
                # combines: den = sum_a F_a*S_a (f32), num likewise (bf16)
                nc.vector.tensor_tensor(out=fs_t, in0=scrF, in1=S_t,
                                        op=mybir.AluOpType.mult)
                nc.vector.tensor_tensor(out=ft_t, in0=scrF, in1=T_t,
                                        op=mybir.AluOpType.mult)
                nc.vector.tensor_tensor(out=dsum, in0=fs_t[:, 0],
                                        in1=fs_t[:, 1], op=mybir.AluOpType.add)
                nc.vector.tensor_tensor(out=den32, in0=dsum, in1=fs_t[:, 2],
                                        op=mybir.AluOpType.add)
                nc.vector.tensor_tensor(out=nsum, in0=ft_t[:, 0],
                                        in1=ft_t[:, 1], op=mybir.AluOpType.add)
                nc.vector.tensor_tensor(out=nsum, in0=nsum, in1=ft_t[:, 2],
                                        op=mybir.AluOpType.add)
                nc.vector.reciprocal_approx_fast(out=rcp32, in_=den32)
                nc.vector.tensor_tensor(
                    out=outt.rearrange("r (d h w) -> r d h w", d=DLOC, h=HB),
                    in0=_dhw_ap(nsum), in1=_dhw_ap(rcp32),
                    op=mybir.AluOpType.mult)
                nc.gpsimd.dma_start(out=y[g], in_=outt)
            rpsum_ctx.__exit__(None, None, None)
    nc.compile()
    return nc


def _host_prep(x, Wq, Wk, Wv, rel_h, rel_w, rel_d):
    import ml_dtypes
    tobf = lambda a: np.ascontiguousarray(a).astype(ml_dtypes.bfloat16)

    x = np.asarray(x, np.float32).reshape(CIN, D, H, W)
    xp = np.pad(x, ((0, 0), (1, 1), (1, 1), (1, 1)))  # (32, 26, 50, 66)
    wqT = np.ascontiguousarray(np.asarray(Wq, np.float32).T)
    wkvT = np.zeros((CIN, 96), np.float32)
    wkvT[:, 0:48] = np.asarray(Wk, np.float32).T
    wkvT[:, 48:96] = np.asarray(Wv, np.float32).T

    # relt[g, r, a]: pass g rows r=(s, c_sub); ch group g, rel axis value a
    rel_d2 = np.asarray(rel_d, np.float32).reshape(C3, K)  # ch 0-15, a=wj
    rel_h2 = np.asarray(rel_h, np.float32).reshape(C3, K)  # ch 16-31, a=dj
    rel_w2 = np.asarray(rel_w, np.float32).reshape(C3, K)  # ch 32-47, a=hj
    relt = np.zeros((NPASS, 128, 3), np.float32)
    csub = np.arange(128) % CG
    relt[0] = rel_d2[csub]
    relt[1] = rel_h2[csub]
    relt[2] = rel_w2[csub]

    eyem = np.eye(128, dtype=np.float32)

    in_maps = []
    for i in range(NCORES):
        slab = xp[:, 3 * i:3 * i + DP]  # (32, 5, 50, 66)
        xb = np.empty((CIN, NS, DP, HBP, WP), np.float32)
        for s in range(NS):
            xb[:, s] = slab[:, :, HB * s:HB * s + HBP, :]
        in_maps.append({
            "xs": tobf(xb), "wq": tobf(wqT), "wkv": tobf(wkvT),
            "relt": relt, "eye": tobf(eyem),
        })
    return in_maps


def kernel(x, Wq, Wk, Wv, rel_h, rel_w, rel_d, trace=False):
    in_maps = _host_prep(x, Wq, Wk, Wv, rel_h, rel_w, rel_d)
    if "nc" not in _CACHE:
        _CACHE["nc"] = build_program()
    res = run_bass_kernel_spmd(
        _CACHE["nc"], in_maps, core_ids=list(range(NCORES)), trace=trace)
    # y per core: (NPASS, 128, NOUT); row r=(s, c_sub) of pass g
    out = np.zeros((COUT, D, H, W), np.float32)
    for i in range(NCORES):
        yv = np.asarray(res.results[i]["y"]).astype(np.float32)
        yv = yv.reshape(NPASS, NS, CG, DLOC, HB, W)
        for g in range(NPASS):
            for s in range(NS):
                out[g * CG:(g + 1) * CG, 3 * i:3 * i + DLOC,
                    HB * s:HB * s + HB, :] = yv[g, s]
    if trace:
        _CACHE["last"] = res
    return out.reshape(1, COUT, D, H, W)


# revision 13
# speedup vs baseline: 1.7945x; 1.1519x over previous
"""Trainium2 Bass kernel for 3D windowed (3x3x3) per-channel softmax attention.

Problem (hardcoded): x (1,32,24,48,64) f32; Wq/Wk/Wv (48,32); rel_* (16,...,3).
  q = Wq@x ; kf/vf = Wk/Wv @ pad(x) ; per (c,voxel): softmax over the 27
  window taps of q*(k_win+rel), then weighted sum of v_win.

Strategy (v2):
  - Shard D=24 across 8 cores (3 output d-slices each + 1-voxel halo,
    zero-padded on host). SPMD, no collectives.
  - Rows r = (s, c_sub): 8 H-blocks x 16 channels = 128 partitions/pass,
    3 passes = one channel GROUP of 16 per pass. Within a pass every
    channel shares the same rel axis (ch 0-15: rel_d varies over wj,
    16-31: rel_h over dj, 32-47: rel_w over hj), so
      exp(q*(k+rel_j)) = exp(q*k_j) * F_a,   F_a = exp(q*rel_a)
    factors with only 3 F planes per pass.
  - qk logits via plain tensor_tensor (2x DVE mode; the baseline's
    scalar_tensor_tensor runs at 1x), batched 3 taps per op via window APs.
  - exp on ACT (in-place over the logit planes).
  - Per-axis sums S_a = sum_{j in a} e_j and T_a = sum_{j in a} e_j*v_j
    either on PE (identity-matmul PSUM accumulation, Pool evicts) or on
    DVE (pairwise trees) - configurable via RED_DEN/RED_NUM.
  - den = sum_a F_a*S_a, num = sum_a F_a*T_a, out = num * recip(den).
  - No DRAM bounce: projections go PSUM -> SBUF stage -> SBUF-SBUF DMA
    gather into per-pass row tiles.
"""

import sys

sys.path.insert(0, "/opt/trn_rl_repo")

import numpy as np

import concourse.bass as bass
import concourse.bacc as bacc
import concourse.mybir as mybir
import concourse.tile as tile
from concourse.bass_utils import run_bass_kernel_spmd

# ---- problem constants (hardcoded per contract) ----
B, CIN, D, H, W = 1, 32, 24, 48, 64
COUT, K, C3 = 48, 3, 16
NCORES = 8
DLOC = D // NCORES            # 3 output d-slices per core
DP = DLOC + 2                 # 5 padded d-planes per core
NS = 8                        # H-blocks per core
HB = H // NS                  # 6 output rows per block
HBP = HB + 2                  # 8 padded rows per block
WP = W + 2                    # 66
BLK = DP * HBP * WP           # 2640 padded voxels per block
SL = HBP * WP                 # 528: one padded d-plane
FL = (HB - 1) * WP + W        # 394-elem flat (h,w) span per d-plane
NV = DLOC * FL                # 1182 per scratch plane
NPASS = 3                     # one channel group per pass
CG = 16                       # channels per group
NJ = 27
NOUT = DLOC * HB * W          # 1152 true output voxels per row

# reduction engines: "pe" (identity matmul accumulate) or "dve" (pair tree)
RED_DEN = "pe"
RED_NUM = "pe"

F32 = mybir.dt.float32
BF16 = mybir.dt.bfloat16

_CACHE = {}


def _tap_geometry(g):
    """Per pass g: list over groups a of list of 3 qk/ev ops.

    Each op is (slot_base, koff, bstride) covering slots
    [slot_base, slot_base+3) with window offsets koff + i*bstride.
    Slot order per pass puts the rel axis outermost (slot = a*9 + o2*3 + i).
    """
    ops = []
    for a in range(3):
        row = []
        for o2 in range(3):
            if g == 0:    # a=wj, o2=dj, batch=hj
                dj, hj, wj, bs = o2, 0, a, WP
            elif g == 1:  # a=dj, o2=hj, batch=wj
                dj, hj, wj, bs = a, o2, 0, 1
            else:         # a=hj, o2=dj, batch=wj
                dj, hj, wj, bs = o2, a, 0, 1
            row.append((a * 9 + o2 * 3, dj * SL + hj * WP + wj, bs))
        ops.append(row)
    return ops


def _win_ap(flat, off, bstride):
    """[128, 3(batch), 3(d), 394] window view into a [128, 2640] tile."""
    base = flat[:, off:off + 1]
    return bass.AP(tensor=base.tensor, offset=base.offset,
                   ap=[base.ap[0], [bstride, 3], [SL, DLOC], [1, FL]])


def _q_ap(qt, rep):
    """[128, rep(broadcast), 3(d), 394] from a [128, NV] q tile."""
    base = qt[:, 0:1]
    return bass.AP(tensor=base.tensor, offset=base.offset,
                   ap=[base.ap[0], [0, rep], [FL, DLOC], [1, FL]])


def _scr_ap(scr, s0, n):
    """[128, n(slots), 3(d), 394] view of scratch slots [s0, s0+n)."""
    base = scr[:, s0, 0:1]
    return bass.AP(tensor=base.tensor, offset=base.offset,
                   ap=[base.ap[0], [NV, n], [FL, DLOC], [1, FL]])


def _dhw_ap(t, col0=0):
    """[128, 3(d), 6(h), 64(w)] true-voxel view of a [128, NV] plane tile."""
    base = t[:, col0:col0 + 1]
    return bass.AP(tensor=base.tensor, offset=base.offset,
                   ap=[base.ap[0], [FL, DLOC], [WP, HB], [1, W]])


def build_program():
    nc = bacc.Bacc("TRN2", target_bir_lowering=False, debug=False,
                   num_devices=NCORES)

    xs = nc.declare_dram_parameter("xs", [CIN, NS, DP, HBP, WP], BF16,
                                   isOutput=False)
    wq = nc.declare_dram_parameter("wq", [CIN, COUT], BF16, isOutput=False)
    wkv = nc.declare_dram_parameter("wkv", [CIN, 96], BF16, isOutput=False)
    relt = nc.declare_dram_parameter("relt", [NPASS, 128, 3], F32,
                                     isOutput=False)
    eye = nc.declare_dram_parameter("eye", [128, 128], BF16, isOutput=False)
    y = nc.declare_dram_parameter("y", [NPASS, 128, NOUT], BF16, isOutput=True)

    # psum column chunks for the PE reductions
    CH = [(0, 394), (394, 394), (788, 394)]

    with tile.TileContext(nc) as tc:
        with (
            tc.tile_pool(name="consts", bufs=1) as consts,
            tc.tile_pool(name="rows", bufs=1) as rows_pool,
            tc.tile_pool(name="attn", bufs=1) as attn,
        ):
            # ---- constants ----
            wq_sb = consts.tile([CIN, COUT], BF16, name="wq_sb")
            nc.sync.dma_start(out=wq_sb, in_=wq[:])
            wkv_sb = consts.tile([CIN, 96], BF16, name="wkv_sb")
            nc.sync.dma_start(out=wkv_sb, in_=wkv[:])
            rel_sb = consts.tile([128, NPASS, 3], F32, name="rel_sb")
            nc.sync.dma_start(out=rel_sb, in_=relt[:].rearrange("p r a -> r p a"))
            eye_sb = consts.tile([128, 128], BF16, name="eye_sb")
            nc.sync.dma_start(out=eye_sb, in_=eye[:])

            # ---- per-pass row tiles (all 3 passes resident) ----
            kf = [rows_pool.tile([128, BLK], BF16, name=f"kf{g}")
                  for g in range(NPASS)]
            vf = [rows_pool.tile([128, BLK], BF16, name=f"vf{g}")
                  for g in range(NPASS)]
            qt = [rows_pool.tile([128, NV], BF16, name=f"qt{g}")
                  for g in range(NPASS)]

            # ---- projection prologue, per s-block ----
            with (
                tc.tile_pool(name="psum", bufs=2, space="PSUM") as psum_pool,
                tc.tile_pool(name="stage", bufs=2) as stage,
            ):
              for s in range(NS):
                xst = stage.tile([CIN, BLK], BF16, tag="xst", name=f"xst{s}")
                nc.sync.dma_start(out=xst, in_=xs[:, s].rearrange(
                    "c d h w -> c (d h w)"))
                kvst = stage.tile([96, BLK], BF16, tag="kvst", name=f"kvst{s}")
                qst = stage.tile([COUT, DLOC, FL], BF16, tag="qst",
                                 name=f"qst{s}")
                for i in range(BLK // 440):
                    ps = psum_pool.tile([96, 440], F32, tag="pskv", name="pskv")
                    nc.tensor.matmul(ps, wkv_sb, xst[:, i * 440:(i + 1) * 440],
                                     start=True, stop=True)
                    nc.vector.tensor_copy(out=kvst[:, i * 440:(i + 1) * 440],
                                          in_=ps)
                # q on the block interior; one matmul per output d-plane
                for d in range(DLOC):
                    psq = psum_pool.tile([COUT, HB * W], F32, tag="psq",
                                         name="psq")
                    rb = xst[:, (d + 1) * SL + WP + 1:(d + 1) * SL + WP + 2]
                    rhs = bass.AP(tensor=rb.tensor, offset=rb.offset,
                                  ap=[rb.ap[0], [WP, HB], [1, W]])
                    nc.tensor.matmul(psq, wq_sb, rhs, start=True, stop=True)
                    qb = qst[:, d, 0:1]
                    qout = bass.AP(tensor=qb.tensor, offset=qb.offset,
                                   ap=[qb.ap[0], [WP, HB], [1, W]])
                    nc.vector.tensor_copy(out=qout, in_=psq)
                # scatter stage rows into per-pass row tiles (SBUF->SBUF DMA)
                r0 = s * CG
                for g in range(NPASS):
                    nc.sync.dma_start(out=kf[g][r0:r0 + CG],
                                      in_=kvst[g * CG:(g + 1) * CG])
                    nc.sync.dma_start(out=vf[g][r0:r0 + CG],
                                      in_=kvst[48 + g * CG:48 + (g + 1) * CG])
                    nc.sync.dma_start(
                        out=qt[g][r0:r0 + CG],
                        in_=qst[g * CG:(g + 1) * CG].rearrange(
                            "c d f -> c (d f)"))

            # ---- attention passes ----
            rpsum_ctx = tc.tile_pool(name="rpsum", bufs=2, space="PSUM")
            rpsum_pool = rpsum_ctx.__enter__()
            # scratch: per-group slot tiles so cross-pass deps stay fine-grained
            scrg = [attn.tile([128, 9, NV], BF16, name=f"scrg{a}")
                    for a in range(3)]
            scrF = attn.tile([128, 3, NV], BF16, name="scrF")
            tmp4 = attn.tile([128, 4, NV], BF16, name="tmp4")
            S_t = attn.tile([128, 3, NV], BF16, name="S_t")
            T_t = attn.tile([128, 3, NV], BF16, name="T_t")
            fs_t = attn.tile([128, 3, NV], BF16, name="fs_t")
            ft_t = attn.tile([128, 3, NV], BF16, name="ft_t")
            dsum = attn.tile([128, NV], BF16, name="dsum")
            nsum = attn.tile([128, NV], BF16, name="nsum")
            den32 = attn.tile([128, NV], F32, name="den32")
            rcp32 = attn.tile([128, NV], F32, name="rcp32")

            def pe_reduce(src_tile, dst, a):
                """dst[:, a] = sum of src_tile's 9 planes (identity matmul)."""
                for (c0, cw) in CH:
                    psr = rpsum_pool.tile([128, cw], F32, tag=f"psr{c0}",
                                          name="psr")
                    for j in range(9):
                        nc.tensor.matmul(
                            psr, eye_sb, src_tile[:, j, c0:c0 + cw],
                            start=(j == 0), stop=(j == 8))
                    nc.scalar.copy(dst[:, a, c0:c0 + cw], psr)

            def dve_reduce(src_tile, dst, a):
                """dst[:, a] = sum of src_tile's 9 planes (pair tree)."""
                nc.vector.tensor_tensor(
                    out=tmp4, in0=src_tile[:, 0:8:2], in1=src_tile[:, 1:8:2],
                    op=mybir.AluOpType.add)
                nc.vector.tensor_tensor(
                    out=tmp4[:, 0:2], in0=tmp4[:, 0:2], in1=tmp4[:, 2:4],
                    op=mybir.AluOpType.add)
                nc.vector.tensor_tensor(
                    out=tmp4[:, 0], in0=tmp4[:, 0], in1=tmp4[:, 1],
                    op=mybir.AluOpType.add)
                nc.vector.tensor_tensor(
                    out=dst[:, a], in0=tmp4[:, 0], in1=src_tile[:, 8],
                    op=mybir.AluOpType.add)

            for g in range(NPASS):
                geo = _tap_geometry(g)
                kff, vff, qtg = kf[g], vf[g], qt[g]
                outt = attn.tile([128, NOUT], BF16, tag="outt", name="outt")

                # logits + exp + den-reduce + ev + num-reduce, per axis group
                for a in range(3):
                    for oi, (sb, koff, bs) in enumerate(geo[a]):
                        nc.vector.tensor_tensor(
                            out=_scr_ap(scrg[a], sb - a * 9, 3),
                            in0=_q_ap(qtg, 3),
                            in1=_win_ap(kff, koff, bs),
                            op=mybir.AluOpType.mult)
                        sub = scrg[a][:, 3 * oi:3 * oi + 3].rearrange(
                            "r j v -> r (j v)")
                        nc.scalar.activation(
                            out=sub, in_=sub,
                            func=mybir.ActivationFunctionType.Exp)
                    nc.vector.tensor_scalar(
                        out=scrF[:, a], in0=qtg,
                        scalar1=rel_sb[:, g, a:a + 1], scalar2=None,
                        op0=mybir.AluOpType.mult)
                    if RED_DEN == "pe":
                        pe_reduce(scrg[a], S_t, a)
                    else:
                        dve_reduce(scrg[a], S_t, a)
                nc.scalar.activation(
                    out=scrF.rearrange("r j v -> r (j v)"),
                    in_=scrF.rearrange("r j v -> r (j v)"),
                    func=mybir.ActivationFunctionType.Exp)
                # e <- e * v_win (in place), then numerator per-axis sums
                for a in range(3):
                    for (sb, koff, bs) in geo[a]:
                        ap = _scr_ap(scrg[a], sb - a * 9, 3)
                        nc.vector.tensor_tensor(
                            out=ap, in0=ap, in1=_win_ap(vff, koff, bs),
                            op=mybir.AluOpType.mult)
                    if RED_NUM == "pe":
                        pe_reduce(scrg[a], T_t, a)
                    else:
                        dve_reduce(scrg[a], T_t, a)
                # combines: den = sum_a F_a*S_a (f32), num likewise (bf16)
                nc.vector.tensor_tensor(out=fs_t, in0=scrF, in1=S_t,
                                        op=mybir.AluOpType.mult)
                nc.vector.tensor_tensor(out=ft_t, in0=scrF, in1=T_t,
                                        op=mybir.AluOpType.mult)
                nc.vector.tensor_tensor(out=dsum, in0=fs_t[:, 0],
                                        in1=fs_t[:, 1], op=mybir.AluOpType.add)
                nc.vector.tensor_tensor(out=den32, in0=dsum, in1=fs_t[:, 2],
                                        op=mybir.AluOpType.add)
                nc.vector.tensor_tensor(out=nsum, in0=ft_t[:, 0],
                                        in1=ft_t[:, 1], op=mybir.AluOpType.add)
                nc.vector.tensor_tensor(out=nsum, in0=nsum, in1=ft_t[:, 2],
                                        op=mybir.AluOpType.add)
                nc.vector.reciprocal_approx_fast(out=rcp32, in_=den32)
                nc.vector.tensor_tensor(
                    out=outt.rearrange("r (d h w) -> r d h w", d=DLOC, h=HB),
                    in0=_dhw_ap(nsum), in1=_dhw_ap(rcp32),
                    op=mybir.AluOpType.mult)
                nc.gpsimd.dma_start(out=y[g], in_=outt)
            rpsum_ctx.__exit__(None, None, None)
    nc.compile()
    return nc


def _host_prep(x, Wq, Wk, Wv, rel_h, rel_w, rel_d):
    import ml_dtypes
    tobf = lambda a: np.ascontiguousarray(a).astype(ml_dtypes.bfloat16)

    x = np.asarray(x, np.float32).reshape(CIN, D, H, W)
    xp = np.pad(x, ((0, 0), (1, 1), (1, 1), (1, 1)))  # (32, 26, 50, 66)
    wqT = np.ascontiguousarray(np.asarray(Wq, np.float32).T)
    wkvT = np.zeros((CIN, 96), np.float32)
    wkvT[:, 0:48] = np.asarray(Wk, np.float32).T
    wkvT[:, 48:96] = np.asarray(Wv, np.float32).T

    # relt[g, r, a]: pass g rows r=(s, c_sub); ch group g, rel axis value a
    rel_d2 = np.asarray(rel_d, np.float32).reshape(C3, K)  # ch 0-15, a=wj
    rel_h2 = np.asarray(rel_h, np.float32).reshape(C3, K)  # ch 16-31, a=dj
    rel_w2 = np.asarray(rel_w, np.float32).reshape(C3, K)  # ch 32-47, a=hj
    relt = np.zeros((NPASS, 128, 3), np.float32)
    csub = np.arange(128) % CG
    relt[0] = rel_d2[csub]
    relt[1] = rel_h2[csub]
    relt[2] = rel_w2[csub]

    eyem = np.eye(128, dtype=np.float32)

    in_maps = []
    for i in range(NCORES):
        slab = xp[:, 3 * i:3 * i + DP]  # (32, 5, 50, 66)
        xb = np.empty((CIN, NS, DP, HBP, WP), np.float32)
        for s in range(NS):
            xb[:, s] = slab[:, :, HB * s:HB * s + HBP, :]
        in_maps.append({
            "xs": tobf(xb), "wq": tobf(wqT), "wkv": tobf(wkvT),
            "relt": relt, "eye": tobf(eyem),
        })
    return in_maps


def kernel(x, Wq, Wk, Wv, rel_h, rel_w, rel_d, trace=False):
    in_maps = _host_prep(x, Wq, Wk, Wv, rel_h, rel_w, rel_d)
    if "nc" not in _CACHE:
        _CACHE["nc"] = build_program()
    res = run_bass_kernel_spmd(
        _CACHE["nc"], in_maps, core_ids=list(range(NCORES)), trace=trace)
    # y per core: (NPASS, 128, NOUT); row r=(s, c_sub) of pass g
    out = np.zeros((COUT, D, H, W), np.float32)
    for i in range(NCORES):
        yv = np.asarray(res.results[i]["y"]).astype(np.float32)
        yv = yv.reshape(NPASS, NS, CG, DLOC, HB, W)
        for g in range(NPASS):
            for s in range(NS):
                out[g * CG:(g + 1) * CG, 3 * i:3 * i + DLOC,
                    HB * s:HB * s + HB, :] = yv[g, s]
    if trace:
        _CACHE["last"] = res
    return out.reshape(1, COUT, D, H, W)


# revision 24
# speedup vs baseline: 1.9907x; 1.1093x over previous
"""Trainium2 Bass kernel for 3D windowed (3x3x3) per-channel softmax attention.

Problem (hardcoded): x (1,32,24,48,64) f32; Wq/Wk/Wv (48,32); rel_* (16,...,3).
  q = Wq@x ; kf/vf = Wk/Wv @ pad(x) ; per (c,voxel): softmax over the 27
  window taps of q*(k_win+rel), then weighted sum of v_win.

Strategy (v2):
  - Shard D=24 across 8 cores (3 output d-slices each + 1-voxel halo,
    zero-padded on host). SPMD, no collectives.
  - Rows r = (s, c_sub): 8 H-blocks x 16 channels = 128 partitions/pass,
    3 passes = one channel GROUP of 16 per pass. Within a pass every
    channel shares the same rel axis (ch 0-15: rel_d varies over wj,
    16-31: rel_h over dj, 32-47: rel_w over hj), so
      exp(q*(k+rel_j)) = exp(q*k_j) * F_a,   F_a = exp(q*rel_a)
    factors with only 3 F planes per pass.
  - qk logits via plain tensor_tensor (2x DVE mode; the baseline's
    scalar_tensor_tensor runs at 1x), batched 3 taps per op via window APs.
  - exp on ACT (in-place over the logit planes).
  - Per-axis sums S_a = sum_{j in a} e_j and T_a = sum_{j in a} e_j*v_j
    either on PE (identity-matmul PSUM accumulation, Pool evicts) or on
    DVE (pairwise trees) - configurable via RED_DEN/RED_NUM.
  - den = sum_a F_a*S_a, num = sum_a F_a*T_a, out = num * recip(den).
  - No DRAM bounce: projections go PSUM -> SBUF stage -> SBUF-SBUF DMA
    gather into per-pass row tiles.
"""

import sys

sys.path.insert(0, "/opt/trn_rl_repo")

import numpy as np

import concourse.bass as bass
import concourse.bacc as bacc
import concourse.mybir as mybir
import concourse.tile as tile
from concourse.bass_utils import run_bass_kernel_spmd

# ---- problem constants (hardcoded per contract) ----
B, CIN, D, H, W = 1, 32, 24, 48, 64
COUT, K, C3 = 48, 3, 16
NCORES = 8
DLOC = D // NCORES            # 3 output d-slices per core
DP = DLOC + 2                 # 5 padded d-planes per core
NS = 8                        # H-blocks per core
HB = H // NS                  # 6 output rows per block
HBP = HB + 2                  # 8 padded rows per block
WP = W + 2                    # 66
BLK = DP * HBP * WP           # 2640 padded voxels per block
SL = HBP * WP                 # 528: one padded d-plane
FL = (HB - 1) * WP + W        # 394-elem flat (h,w) span per d-plane
NV = DLOC * FL                # 1182 per scratch plane
NPASS = 3                     # one channel group per pass
CG = 16                       # channels per group
NJ = 27
NOUT = DLOC * HB * W          # 1152 true output voxels per row

# reduction engines: "pe" (identity matmul accumulate) or "dve" (pair tree)
RED_DEN = "pe"
RED_NUM = "pe"

F32 = mybir.dt.float32
BF16 = mybir.dt.bfloat16

_CACHE = {}


def _tap_geometry(g):
    """Per pass g: list over groups a of list of 3 qk/ev ops.

    Each op is (slot_base, koff, bstride) covering slots
    [slot_base, slot_base+3) with window offsets koff + i*bstride.
    Slot order per pass puts the rel axis outermost (slot = a*9 + o2*3 + i).
    """
    ops = []
    for a in range(3):
        row = []
        for o2 in range(3):
            if g == 0:    # a=wj, o2=dj, batch=hj
                dj, hj, wj, bs = o2, 0, a, WP
            elif g == 1:  # a=dj, o2=hj, batch=wj
                dj, hj, wj, bs = a, o2, 0, 1
            else:         # a=hj, o2=dj, batch=wj
                dj, hj, wj, bs = o2, a, 0, 1
            row.append((a * 9 + o2 * 3, dj * SL + hj * WP + wj, bs))
        ops.append(row)
    return ops


def _win_ap(flat, off, bstride):
    """[128, 3(batch), 3(d), 394] window view into a [128, 2640] tile."""
    base = flat[:, off:off + 1]
    return bass.AP(tensor=base.tensor, offset=base.offset,
                   ap=[base.ap[0], [bstride, 3], [SL, DLOC], [1, FL]])


def _q_ap(qt, rep):
    """[128, rep(broadcast), 3(d), 394] from a [128, NV] q tile."""
    base = qt[:, 0:1]
    return bass.AP(tensor=base.tensor, offset=base.offset,
                   ap=[base.ap[0], [0, rep], [FL, DLOC], [1, FL]])


def _scr_ap(scr, s0, n):
    """[128, n(slots), 3(d), 394] view of scratch slots [s0, s0+n)."""
    base = scr[:, s0, 0:1]
    return bass.AP(tensor=base.tensor, offset=base.offset,
                   ap=[base.ap[0], [NV, n], [FL, DLOC], [1, FL]])


def _dhw_ap(t, col0=0):
    """[128, 3(d), 6(h), 64(w)] true-voxel view of a [128, NV] plane tile."""
    base = t[:, col0:col0 + 1]
    return bass.AP(tensor=base.tensor, offset=base.offset,
                   ap=[base.ap[0], [FL, DLOC], [WP, HB], [1, W]])


def build_program():
    nc = bacc.Bacc("TRN2", target_bir_lowering=False, debug=False,
                   num_devices=NCORES)

    xs = nc.declare_dram_parameter("xs", [CIN, NS, DP, HBP, WP], BF16,
                                   isOutput=False)
    wq = nc.declare_dram_parameter("wq", [CIN, COUT], BF16, isOutput=False)
    wkv = nc.declare_dram_parameter("wkv", [CIN, 96], BF16, isOutput=False)
    relt = nc.declare_dram_parameter("relt", [NPASS, 128, 3], F32,
                                     isOutput=False)
    eye = nc.declare_dram_parameter("eye", [128, 128], BF16, isOutput=False)
    y = nc.declare_dram_parameter("y", [NPASS, 128, NOUT], BF16, isOutput=True)

    # psum column chunks for the PE reductions
    CH = [(0, 394), (394, 394), (788, 394)]

    with tile.TileContext(nc) as tc:
        with (
            tc.tile_pool(name="consts", bufs=1) as consts,
            tc.tile_pool(name="rows", bufs=1) as rows_pool,
            tc.tile_pool(name="attn", bufs=1) as attn,
        ):
            # ---- constants ----
            wq_sb = consts.tile([CIN, COUT], BF16, name="wq_sb")
            nc.sync.dma_start(out=wq_sb, in_=wq[:])
            wkv_sb = consts.tile([CIN, 96], BF16, name="wkv_sb")
            nc.sync.dma_start(out=wkv_sb, in_=wkv[:])
            rel_sb = consts.tile([128, NPASS, 3], F32, name="rel_sb")
            nc.sync.dma_start(out=rel_sb, in_=relt[:].rearrange("p r a -> r p a"))
            eye_sb = consts.tile([128, 128], BF16, name="eye_sb")
            nc.sync.dma_start(out=eye_sb, in_=eye[:])

            # ---- per-pass row tiles (all 3 passes resident) ----
            # kvf[g][:, 0] = k rows, kvf[g][:, 1] = v rows
            kvf = [rows_pool.tile([128, 2, BLK], BF16, name=f"kvf{g}")
                   for g in range(NPASS)]
            qt = [rows_pool.tile([128, NV], BF16, name=f"qt{g}")
                  for g in range(NPASS)]

            # ---- projection prologue, per s-block ----
            with (
                tc.tile_pool(name="psum", bufs=2, space="PSUM") as psum_pool,
                tc.tile_pool(name="stage", bufs=2) as stage,
            ):
              for s in range(NS):
                xst = stage.tile([CIN, BLK], BF16, tag="xst", name=f"xst{s}")
                dmae = nc.sync if s % 2 == 0 else nc.scalar
                dmae.dma_start(out=xst, in_=xs[:, s].rearrange(
                    "c d h w -> c (d h w)"))
                kvst = stage.tile([96, BLK], BF16, tag="kvst", name=f"kvst{s}")
                qst = stage.tile([COUT, DLOC, FL], BF16, tag="qst",
                                 name=f"qst{s}")
                for i in range(BLK // 440):
                    ps = psum_pool.tile([96, 440], F32, tag="pskv", name="pskv")
                    nc.tensor.matmul(ps, wkv_sb, xst[:, i * 440:(i + 1) * 440],
                                     start=True, stop=True)
                    if i % 2 == 0:
                        nc.vector.tensor_copy(
                            out=kvst[:, i * 440:(i + 1) * 440], in_=ps)
                    else:
                        nc.scalar.copy(kvst[:, i * 440:(i + 1) * 440], ps)
                # q on the block interior; one matmul per output d-plane
                for d in range(DLOC):
                    psq = psum_pool.tile([COUT, HB * W], F32, tag="psq",
                                         name="psq")
                    rb = xst[:, (d + 1) * SL + WP + 1:(d + 1) * SL + WP + 2]
                    rhs = bass.AP(tensor=rb.tensor, offset=rb.offset,
                                  ap=[rb.ap[0], [WP, HB], [1, W]])
                    nc.tensor.matmul(psq, wq_sb, rhs, start=True, stop=True)
                    qb = qst[:, d, 0:1]
                    qout = bass.AP(tensor=qb.tensor, offset=qb.offset,
                                   ap=[qb.ap[0], [WP, HB], [1, W]])
                    if d % 2 == 0:
                        nc.vector.tensor_copy(out=qout, in_=psq)
                    else:
                        nc.scalar.copy(qout, psq)
                # scatter stage rows into per-pass row tiles (SBUF->SBUF DMA).
                # wkv cols are (c, kv)-interleaved per group, so one DMA moves
                # k+v: src partitions (2c, 2c+1) -> dst row r0+c slots (0, 1).
                r0 = s * CG
                for g in range(NPASS):
                    dmae = nc.sync if (s + g) % 2 == 0 else nc.scalar
                    dmae.dma_start(out=kvf[g][r0:r0 + CG, 0],
                                   in_=kvst[g * CG:(g + 1) * CG])
                    dmae.dma_start(out=kvf[g][r0:r0 + CG, 1],
                                   in_=kvst[48 + g * CG:48 + (g + 1) * CG])
                    dmae.dma_start(
                        out=qt[g][r0:r0 + CG],
                        in_=qst[g * CG:(g + 1) * CG].rearrange(
                            "c d f -> c (d f)"))

            # ---- attention passes ----
            rpsum_ctx = tc.tile_pool(name="rpsum", bufs=1, space="PSUM")
            rpsum_pool = rpsum_ctx.__enter__()
            # scratch: per-group slot tiles so cross-pass deps stay fine-grained
            scrg = [attn.tile([128, 9, NV], BF16, name=f"scrg{a}")
                    for a in range(3)]
            nsum = attn.tile([128, NV], BF16, name="nsum")
            den32 = attn.tile([128, NV], F32, name="den32")
            rcp32 = attn.tile([128, NV], F32, name="rcp32")

            for g in range(NPASS):
                geo = _tap_geometry(g)
                kff, vff, qtg = kvf[g][:, 0], kvf[g][:, 1], qt[g]
                outt = attn.tile([128, NOUT], BF16, tag="outt", name="outt")
                psd = [rpsum_pool.tile([128, cw], F32, tag=f"psd{c0}",
                                       name="psd") for (c0, cw) in CH]
                psn = [rpsum_pool.tile([128, cw], F32, tag=f"psn{c0}",
                                       name="psn") for (c0, cw) in CH]

                # rel folded into k: ka = k + rel_a (per-partition scalar),
                # so e = exp(q*ka) needs no separate rel factor downstream.
                for a in range(3):
                    ka = attn.tile([128, BLK], BF16, tag="ka", name="ka")
                    nc.vector.tensor_scalar(
                        out=ka, in0=kff, scalar1=rel_sb[:, g, a:a + 1],
                        scalar2=None, op0=mybir.AluOpType.add)
                    for oi, (sb, koff, bs) in enumerate(geo[a]):
                        nc.vector.tensor_tensor(
                            out=_scr_ap(scrg[a], sb - a * 9, 3),
                            in0=_q_ap(qtg, 3),
                            in1=_win_ap(ka, koff, bs),
                            op=mybir.AluOpType.mult)
                        sub = scrg[a][:, 3 * oi:3 * oi + 3].rearrange(
                            "r j v -> r (j v)")
                        nc.scalar.activation(
                            out=sub, in_=sub,
                            func=mybir.ActivationFunctionType.Exp)
                    # accumulate denominator: all 27 planes into one psum/chunk
                    for ci, (c0, cw) in enumerate(CH):
                        for j in range(9):
                            nc.tensor.matmul(
                                psd[ci], eye_sb, scrg[a][:, j, c0:c0 + cw],
                                start=(a == 0 and j == 0),
                                stop=(a == 2 and j == 8))
                # e <- e * v_win (in place), accumulate numerator
                for a in range(3):
                    for (sb, koff, bs) in geo[a]:
                        ap = _scr_ap(scrg[a], sb - a * 9, 3)
                        nc.vector.tensor_tensor(
                            out=ap, in0=ap, in1=_win_ap(vff, koff, bs),
                            op=mybir.AluOpType.mult)
                    for ci, (c0, cw) in enumerate(CH):
                        for j in range(9):
                            nc.tensor.matmul(
                                psn[ci], eye_sb, scrg[a][:, j, c0:c0 + cw],
                                start=(a == 0 and j == 0),
                                stop=(a == 2 and j == 8))
                # evict sums, reciprocal, final
                for ci, (c0, cw) in enumerate(CH):
                    nc.scalar.copy(den32[:, c0:c0 + cw], psd[ci])
                    nc.scalar.copy(nsum[:, c0:c0 + cw], psn[ci])
                nc.vector.reciprocal_approx_fast(out=rcp32, in_=den32)
                nc.vector.tensor_tensor(
                    out=outt.rearrange("r (d h w) -> r d h w", d=DLOC, h=HB),
                    in0=_dhw_ap(nsum), in1=_dhw_ap(rcp32),
                    op=mybir.AluOpType.mult)
                nc.gpsimd.dma_start(out=y[g], in_=outt)
            rpsum_ctx.__exit__(None, None, None)
    nc.compile()
    return nc


def _host_prep(x, Wq, Wk, Wv, rel_h, rel_w, rel_d):
    import ml_dtypes
    tobf = lambda a: np.ascontiguousarray(a).astype(ml_dtypes.bfloat16)

    x = np.asarray(x, np.float32).reshape(CIN, D, H, W)
    xp = np.pad(x, ((0, 0), (1, 1), (1, 1), (1, 1)))  # (32, 26, 50, 66)
    wqT = np.ascontiguousarray(np.asarray(Wq, np.float32).T)
    wkvT = np.zeros((CIN, 96), np.float32)
    wkvT[:, 0:48] = np.asarray(Wk, np.float32).T
    wkvT[:, 48:96] = np.asarray(Wv, np.float32).T

    # relt[g, r, a]: pass g rows r=(s, c_sub); ch group g, rel axis value a
    rel_d2 = np.asarray(rel_d, np.float32).reshape(C3, K)  # ch 0-15, a=wj
    rel_h2 = np.asarray(rel_h, np.float32).reshape(C3, K)  # ch 16-31, a=dj
    rel_w2 = np.asarray(rel_w, np.float32).reshape(C3, K)  # ch 32-47, a=hj
    relt = np.zeros((NPASS, 128, 3), np.float32)
    csub = np.arange(128) % CG
    relt[0] = rel_d2[csub]
    relt[1] = rel_h2[csub]
    relt[2] = rel_w2[csub]

    eyem = np.eye(128, dtype=np.float32)

    in_maps = []
    for i in range(NCORES):
        slab = xp[:, 3 * i:3 * i + DP]  # (32, 5, 50, 66)
        xb = np.empty((CIN, NS, DP, HBP, WP), np.float32)
        for s in range(NS):
            xb[:, s] = slab[:, :, HB * s:HB * s + HBP, :]
        in_maps.append({
            "xs": tobf(xb), "wq": tobf(wqT), "wkv": tobf(wkvT),
            "relt": relt, "eye": tobf(eyem),
        })
    return in_maps


def kernel(x, Wq, Wk, Wv, rel_h, rel_w, rel_d, trace=False):
    in_maps = _host_prep(x, Wq, Wk, Wv, rel_h, rel_w, rel_d)
    if "nc" not in _CACHE:
        _CACHE["nc"] = build_program()
    res = run_bass_kernel_spmd(
        _CACHE["nc"], in_maps, core_ids=list(range(NCORES)), trace=trace)
    # y per core: (NPASS, 128, NOUT); row r=(s, c_sub) of pass g
    out = np.zeros((COUT, D, H, W), np.float32)
    for i in range(NCORES):
        yv = np.asarray(res.results[i]["y"]).astype(np.float32)
        yv = yv.reshape(NPASS, NS, CG, DLOC, HB, W)
        for g in range(NPASS):
            for s in range(NS):
                out[g * CG:(g + 1) * CG, 3 * i:3 * i + DLOC,
                    HB * s:HB * s + HB, :] = yv[g, s]
    if trace:
        _CACHE["last"] = res
    return out.reshape(1, COUT, D, H, W)
